# revision 1
# baseline (speedup 1.0000x reference)
"""HOIContactLoss on Trainium2 — pure data-parallel over batch (2 items/core x 8 cores).

Per item, the pairwise squared distances d2[i,j] = |x_i|^2 + |y_j|^2 - 2 x_i.y_j
are produced directly by the TensorEngine via a K=13 bf16 "lifted feature"
matmul: hi/lo bf16 splits of the coordinates recover fp32-level accuracy, and
extra rank-1 rows carry |x|^2, |y|^2 and a +BIG mask for invalid/padded points.
ScalarE relu-drains PSUM to fp16 SBUF tiles; VectorE computes cham_y with a
running elementwise min across x-tiles (+ PE-transpose partition-min) and
cham_x with a fold-tree row-min.  Weighted means are computed on device; the
final scalar mean over the batch is taken on host after gathering 8 cores.
"""
import numpy as np
import ml_dtypes

import concourse.bacc as bacc
import concourse.tile as tile
from concourse import mybir
from concourse.bass_utils import run_bass_kernel_spmd
from contextlib import ExitStack

F32, F16, BF16 = mybir.dt.float32, mybir.dt.float16, mybir.dt.bfloat16
AOP = mybir.AluOpType
ACTF = mybir.ActivationFunctionType

B, P1, P2, D = 16, 6890, 4000, 3
P1P, P2P = 6912, 4096          # padded sizes
NT = P1P // 128                # 54 x-tiles of 128 points
BIG = 30000.0                  # "infinity" that stays finite in fp16 even doubled
N_CORES = 8
IPC = B // N_CORES             # items per core

_compiled = None


def _build():
    nc = bacc.Bacc(None, target_bir_lowering=False)
    with tile.TileContext(nc) as tc:
        with ExitStack() as ctx:
            dram = ctx.enter_context(tc.tile_pool(name="dram", bufs=1, space="DRAM"))
            const = ctx.enter_context(tc.tile_pool(name="const", bufs=1))
            io = ctx.enter_context(tc.tile_pool(name="io", bufs=2))
            acc = ctx.enter_context(tc.tile_pool(name="acc", bufs=2))
            d2p = ctx.enter_context(tc.tile_pool(name="d2p", bufs=3))
            foldp = ctx.enter_context(tc.tile_pool(name="foldp", bufs=2))
            ppool = ctx.enter_context(tc.tile_pool(name="ppool", bufs=2, space="PSUM"))
            spool = ctx.enter_context(tc.tile_pool(name="spool", bufs=2, space="PSUM"))

            xf_d = dram.tile([IPC, 13, P1P], BF16, kind="ExternalInput")
            yf_d = dram.tile([IPC, 13, P2P], BF16, kind="ExternalInput")
            sm_d = dram.tile([IPC, 128, NT], F32, kind="ExternalInput")
            om_d = dram.tile([IPC, 128, 32], F32, kind="ExternalInput")
            idn_d = dram.tile([128, 128], F16, kind="ExternalInput")
            loss_d = dram.tile([IPC, 1], F32, kind="ExternalOutput")

            idn = const.tile([128, 128], F16)
            nc.sync.dma_start(out=idn[:], in_=idn_d[:])
            ones128 = const.tile([128, 1], F32)
            nc.vector.memset(ones128[:], 1.0)

            for it in range(IPC):
                xf = io.tile([13, P1P], BF16, tag="xf")
                nc.sync.dma_start(out=xf[:], in_=xf_d[it])
                yf = io.tile([13, P2P], BF16, tag="yf")
                nc.sync.dma_start(out=yf[:], in_=yf_d[it])
                smap = io.tile([128, NT], F32, tag="smap")
                nc.sync.dma_start(out=smap[:], in_=sm_d[it])
                omap = io.tile([128, 32], F32, tag="omap")
                nc.sync.dma_start(out=omap[:], in_=om_d[it])

                rminY = acc.tile([128, P2P], F16, tag="rminY")
                nc.vector.memset(rminY[:], BIG)
                chamX = acc.tile([128, NT], F32, tag="chamX")
                chamX128 = acc.tile([128, NT, 128], F16, tag="chamX128")

                for t in range(NT):
                    lhsT = xf[:, t * 128:(t + 1) * 128]
                    pgA = ppool.tile([128, 1536], F32, tag="pg", name=f"pgA_{it}_{t}")
                    pgB = ppool.tile([128, 1536], F32, tag="pg", name=f"pgB_{it}_{t}")
                    pgC = ppool.tile([128, 1024], F32, tag="pg", name=f"pgC_{it}_{t}")
                    for c in range(3):
                        nc.tensor.matmul(pgA[:, c * 512:(c + 1) * 512], lhsT,
                                         yf[:, c * 512:(c + 1) * 512],
                                         start=True, stop=True)
                    for c in range(3):
                        nc.tensor.matmul(pgB[:, c * 512:(c + 1) * 512], lhsT,
                                         yf[:, (c + 3) * 512:(c + 4) * 512],
                                         start=True, stop=True)
                    for c in range(2):
                        nc.tensor.matmul(pgC[:, c * 512:(c + 1) * 512], lhsT,
                                         yf[:, (c + 6) * 512:(c + 7) * 512],
                                         start=True, stop=True)

                    d2w = d2p.tile([128, P2P], F16, tag="d2w", name=f"d2w_{it}_{t}")
                    nc.scalar.activation(out=d2w[:, 0:1536], in_=pgA[:], func=ACTF.Relu)
                    nc.scalar.activation(out=d2w[:, 1536:3072], in_=pgB[:], func=ACTF.Relu)
                    nc.scalar.activation(out=d2w[:, 3072:4096], in_=pgC[:], func=ACTF.Relu)

                    # cham_y: running elementwise min across x-tiles
                    nc.vector.tensor_tensor(rminY[:], d2w[:], rminY[:], op=AOP.min)

                    # cham_x: fold tree 4096 -> 128, batched final reduce later
                    f1 = foldp.tile([128, 2048], F16, tag="f1", name=f"f1_{it}_{t}")
                    nc.vector.tensor_tensor(f1[:], d2w[:, 0:2048], d2w[:, 2048:4096], op=AOP.min)
                    nc.vector.tensor_tensor(f1[:, 0:1024], f1[:, 0:1024], f1[:, 1024:2048], op=AOP.min)
                    nc.vector.tensor_tensor(f1[:, 0:512], f1[:, 0:512], f1[:, 512:1024], op=AOP.min)
                    nc.vector.tensor_tensor(f1[:, 0:256], f1[:, 0:256], f1[:, 256:512], op=AOP.min)
                    nc.vector.tensor_tensor(chamX128[:, t, :], f1[:, 0:128], f1[:, 128:256], op=AOP.min)

                # cham_x: one batched 3D reduce over the stashed per-tile folds
                nc.vector.tensor_reduce(out=chamX[:], in_=chamX128[:],
                                        axis=mybir.AxisListType.X, op=AOP.min)

                # cham_y: PE-transpose 128-col slices, reduce 4 slices at a time
                chamYt = acc.tile([128, 32], F32, tag="chamYt")
                for k in range(0, 32, 4):
                    pst = spool.tile([128, 4, 128], F16, tag="pst", name=f"pst_{it}_{k}")
                    for q in range(4):
                        nc.tensor.transpose(pst[:, q, :], rminY[:, (k + q) * 128:(k + q + 1) * 128], idn[:])
                    nc.vector.tensor_reduce(out=chamYt[:, k:k + 4], in_=pst[:],
                                            axis=mybir.AxisListType.X, op=AOP.min)

                # weighted sums -> per-item loss
                vals = acc.tile([128, 4], F32, tag="vals")
                wx = acc.tile([128, NT], F32, tag="wx")
                nc.vector.tensor_tensor(wx[:], chamX[:], smap[:], op=AOP.mult)
                nc.vector.tensor_reduce(out=vals[:, 0:1], in_=wx[:], axis=mybir.AxisListType.X, op=AOP.add)
                wy = acc.tile([128, 32], F32, tag="wy")
                nc.vector.tensor_tensor(wy[:], chamYt[:], omap[:], op=AOP.mult)
                nc.vector.tensor_reduce(out=vals[:, 1:2], in_=wy[:], axis=mybir.AxisListType.X, op=AOP.add)
                nc.vector.tensor_reduce(out=vals[:, 2:3], in_=smap[:], axis=mybir.AxisListType.X, op=AOP.add)
                nc.vector.tensor_reduce(out=vals[:, 3:4], in_=omap[:], axis=mybir.AxisListType.X, op=AOP.add)

                ploss = spool.tile([1, 4], F32, tag="pst", name=f"ploss_{it}")
                nc.tensor.matmul(ploss[:], ones128[:], vals[:], start=True, stop=True)
                lv = acc.tile([1, 4], F32, tag="lv")
                nc.vector.tensor_copy(out=lv[:], in_=ploss[:])
                nc.vector.tensor_scalar_add(lv[:, 2:4], lv[:, 2:4], 1e-6)
                nc.vector.reciprocal(out=lv[:, 2:4], in_=lv[:, 2:4])
                lr = acc.tile([1, 2], F32, tag="lr")
                nc.vector.tensor_tensor(lr[:], lv[:, 0:2], lv[:, 2:4], op=AOP.mult)
                litem = acc.tile([1, 1], F32, tag="litem")
                nc.vector.tensor_reduce(out=litem[:], in_=lr[:], axis=mybir.AxisListType.X, op=AOP.add)
                nc.sync.dma_start(out=loss_d[it], in_=litem[:])

            names = dict(xf=xf_d.name, yf=yf_d.name, sm=sm_d.name, om=om_d.name,
                         idn=idn_d.name, loss=loss_d.name)
    nc.compile()
    return nc, names


def _bf16(a):
    return a.astype(ml_dtypes.bfloat16)


def _prep_item(x, y, sm, om, n):
    """Build lifted-feature tensors for one batch item (host-side repacking)."""
    xx = np.zeros((P1P, 3), np.float32); xx[:P1] = x
    yy = np.zeros((P2P, 3), np.float32); yy[:P2] = y
    x2 = (xx * xx).sum(-1); x2[P1:] = BIG
    y2 = (yy * yy).sum(-1)
    mask = (np.arange(P2P) >= n).astype(np.float32) * BIG
    y2m = y2 + mask
    t = -2.0 * yy
    xh = _bf16(xx); xl = _bf16(xx - xh.astype(np.float32))
    th = _bf16(t);  tl = _bf16(t - th.astype(np.float32))
    x2h = _bf16(x2); x2l = _bf16(x2 - x2h.astype(np.float32))
    y2mh = _bf16(y2m); y2ml = _bf16(y2m - y2mh.astype(np.float32))
    o1 = np.ones(P1P, ml_dtypes.bfloat16); o2 = np.ones(P2P, ml_dtypes.bfloat16)
    XF = np.stack([xh[:, 0], xh[:, 1], xh[:, 2], xl[:, 0], xl[:, 1], xl[:, 2],
                   xh[:, 0], xh[:, 1], xh[:, 2], x2h, x2l, o1, o1])
    YF = np.stack([th[:, 0], th[:, 1], th[:, 2], th[:, 0], th[:, 1], th[:, 2],
                   tl[:, 0], tl[:, 1], tl[:, 2], o2, o2, y2mh, y2ml])
    smp = np.zeros(P1P, np.float32); smp[:P1] = sm[:, 0]
    omp = np.zeros(P2P, np.float32)
    omp[:P2] = np.where(np.arange(P2) < n, om[:, 0], 0.0)
    SM = smp.reshape(NT, 128).T.copy()          # [128, 54] partition-major
    OM = omp.reshape(32, 128).T.copy()          # [128, 32] partition-major
    return XF, YF, SM, OM


def kernel(smpl_v, object_v, smpl_contact_maps, object_contact_maps, object_verts_n,
           trace=False):
    global _compiled
    if _compiled is None:
        _compiled = _build()
    nc, names = _compiled

    smpl_v = np.asarray(smpl_v, np.float32)
    object_v = np.asarray(object_v, np.float32)
    smpl_contact_maps = np.asarray(smpl_contact_maps, np.float32)
    object_contact_maps = np.asarray(object_contact_maps, np.float32)
    ns = np.asarray(object_verts_n).astype(np.int64)

    idn = np.eye(128, dtype=np.float16)
    in_maps = []
    for c in range(N_CORES):
        XFs, YFs, SMs, OMs = [], [], [], []
        for k in range(IPC):
            b = c * IPC + k
            XF, YF, SM, OM = _prep_item(smpl_v[b], object_v[b], smpl_contact_maps[b],
                                        object_contact_maps[b], int(ns[b]))
            XFs.append(XF); YFs.append(YF); SMs.append(SM); OMs.append(OM)
        in_maps.append({
            names['xf']: np.stack(XFs), names['yf']: np.stack(YFs),
            names['sm']: np.stack(SMs), names['om']: np.stack(OMs),
            names['idn']: idn,
        })
    res = run_bass_kernel_spmd(nc, in_maps, core_ids=list(range(N_CORES)), trace=trace)
    losses = np.concatenate([res.results[c][names['loss']][:, 0] for c in range(N_CORES)])
    out = np.float32(losses.mean())
    if trace:
        return out, res
    return out

